# revision 1
# baseline (speedup 1.0000x reference)
"""Bipartite GCN message-passing kernel for 8 Trainium2 NeuronCores.

Math (reference): rst = deg_in^-1/2 * segsum_dst( (node_f @ W_side) * deg_out^-1/2 [src] )
Refactor used here (projection is linear, graph strictly bipartite):
    rst[d] = ( sum_{e->d} c_e * f_raw[src_e] ) @ W_side(d),
    c_e = deg_out[src]^-1/2 * deg_in[dst]^-1/2  (folded on host into scatter tiles)

Sharding: dst nodes dealt round-robin (degree-sorted) to 8 cores -> identical
compile-time schedule per core (SPMD), no collectives. Per core the device:
  1. dma_gather raw fp32 feature rows by src (512B rows, line-rate)
  2. scatter-matmul: PSUM[128f, 512slot] += M_chunk[128e,128f].T @ S_chunk[128e,w]
     where S carries c_e at (edge_row, dst_col) - streamed from host
  3. projection matmul with the side weight, fp32
  4. feat-major output [128, slots]; host transposes/unpermutes.
"""
import sys
import os

for _p in ("/opt/trn_rl_repo",):
    if _p not in sys.path and os.path.isdir(_p):
        sys.path.insert(0, _p)

import numpy as np

N_U = 50000
N_V = 50000
N = N_U + N_V
D = 128
E = 1600000
N_CORES = 8
HALF = 25000          # int16-safe table window
WIN = 512             # dst slots per PSUM window
P = 128


# ----------------------------------------------------------------- host layout
def _build_layout(src, dst, cout, cin):
    """Canonical schedule + per-core edge/scatter data.

    Returns (schedule, per_core), where schedule is compile-time (identical
    across cores) and per_core holds idx/S arrays + output dst mapping.
    """
    layout_phases = []
    per_core_idx = [[] for _ in range(N_CORES)]
    per_core_sval = [[] for _ in range(N_CORES)]   # aligned with idx positions
    per_core_dsts = [[] for _ in range(N_CORES)]   # slot -> global dst id (-1 pad)

    for phase in range(2):
        if phase == 0:       # dsts are v-nodes, sources u-side
            mask = dst >= N_U
            d_local = dst[mask] - N_U
            s_local = src[mask]
            dst_base = N_U
        else:                # dsts are u-nodes, sources v-side
            mask = dst < N_U
            d_local = dst[mask]
            s_local = src[mask] - N_U
            dst_base = 0
        half = (s_local >= HALF).astype(np.int64)
        s_half_local = s_local - half * HALF

        n_dst = N_U
        a_cnt = np.bincount(d_local[half == 0], minlength=n_dst)
        b_cnt = np.bincount(d_local[half == 1], minlength=n_dst)

        order = np.lexsort((np.arange(n_dst), b_cnt, a_cnt))
        rank = np.empty(n_dst, np.int64)
        rank[order] = np.arange(n_dst)

        slots_per_core = (n_dst + N_CORES - 1) // N_CORES  # 6250
        # canonical per-slot degrees = max over cores (clipped >= 1)
        a_mat = np.zeros((N_CORES, slots_per_core), np.int64)
        b_mat = np.zeros((N_CORES, slots_per_core), np.int64)
        dst_mat = np.full((N_CORES, slots_per_core), -1, np.int64)
        r = np.arange(n_dst)
        a_mat[r % N_CORES, r // N_CORES] = a_cnt[order]
        b_mat[r % N_CORES, r // N_CORES] = b_cnt[order]
        dst_mat[r % N_CORES, r // N_CORES] = order + dst_base
        A = np.maximum(a_mat.max(axis=0), 1)
        B = np.maximum(b_mat.max(axis=0), 1)

        for k in range(N_CORES):
            per_core_dsts[k].append(dst_mat[k])

        # ---- canonical chunking per (window, pass), no slot straddles a chunk
        n_win = (slots_per_core + WIN - 1) // WIN
        windows = []
        # canonical edge-position base per slot, per pass
        pos_base = [np.zeros(slots_per_core, np.int64) for _ in (0, 1)]
        for w in range(n_win):
            s0, s1 = w * WIN, min((w + 1) * WIN, slots_per_core)
            wininfo = {"n_slots": s1 - s0, "passes": []}
            for p_i, C in enumerate((A, B)):
                chunks = []   # (col0, w, scol0)
                cur = 0       # fill in current chunk
                cur_chunk = None
                blocks = 0
                for s in range(s0, s1):
                    c = int(C[s])
                    if cur_chunk is None or cur + c > P:
                        if cur_chunk is not None:
                            chunks.append(cur_chunk)
                        cur_chunk = {"col0": s - s0, "cols": 0}
                        blocks += 1
                        cur = 0
                    pos_base[p_i][s] = (blocks - 1) * P + cur
                    cur += c
                    cur_chunk["cols"] = (s - s0) - cur_chunk["col0"] + 1
                if cur_chunk is not None:
                    chunks.append(cur_chunk)
                wininfo["passes"].append({"chunks": chunks, "n_blocks": blocks})
            windows.append(wininfo)
        layout_phases.append({
            "n_win": n_win,
            "slots_per_core": slots_per_core,
            "windows": windows,
        })

        # ---- per-core edge placement (vectorized)
        # rank within (dst, half) group:
        grp = d_local * 2 + half
        sort_i = np.argsort(grp, kind="stable")
        grp_s = grp[sort_i]
        starts = np.r_[0, np.nonzero(np.diff(grp_s))[0] + 1]
        group_start_per_edge = np.empty(len(grp_s), np.int64)
        group_id = np.cumsum(np.r_[0, (np.diff(grp_s) != 0).astype(np.int64)])
        first_pos_of_group = starts[group_id]
        within = np.arange(len(grp_s)) - first_pos_of_group
        e_rank = np.empty(len(grp), np.int64)
        e_rank[sort_i] = within

        e_core = rank[d_local] % N_CORES
        e_slot = rank[d_local] // N_CORES
        e_win = e_slot // WIN

        # global canonical position of each edge within its (win, pass) stream:
        e_pos = np.where(half == 0,
                         pos_base[0][e_slot],
                         pos_base[1][e_slot]) + e_rank

        # canonical call sizes (blocks) per (win, pass):
        call_blocks = np.array(
            [[windows[w]["passes"][p]["n_blocks"] for p in (0, 1)]
             for w in range(n_win)], np.int64)
        # canonical flat offsets: order = win-major, pass lo then hi
        call_sizes = (call_blocks * P).reshape(-1)           # [n_win*2]
        call_off = np.r_[0, np.cumsum(call_sizes)][:-1].reshape(n_win, 2)
        tot_idx = int(call_sizes.sum())

        # canonical S layout: per chunk scol0
        s_cols_per_call = []
        for w in range(n_win):
            for p_i in (0, 1):
                ch = windows[w]["passes"][p_i]["chunks"]
                cols = np.array([c["cols"] for c in ch], np.int64)
                s_cols_per_call.append(cols)
        chunk_cols_flat = np.concatenate(s_cols_per_call)
        chunk_scol0 = np.r_[0, np.cumsum(chunk_cols_flat)][:-1]
        tot_scols = int(chunk_cols_flat.sum())
        # record scol0 / col0 back into schedule for device build
        # (scol0 made global across phases via scol_phase_base)
        scol_phase_base = sum(
            pc.shape[1] for pc in per_core_sval[0]
        ) if per_core_sval[0] else 0
        ci = 0
        for w in range(n_win):
            for p_i in (0, 1):
                for c in windows[w]["passes"][p_i]["chunks"]:
                    c["scol0"] = int(chunk_scol0[ci]) + scol_phase_base
                    ci += 1

        # per-chunk col0 arrays for edge->scol math
        chunk_col0_flat = np.concatenate(
            [np.array([c["col0"] for c in windows[w]["passes"][p_i]["chunks"]],
                      np.int64)
             for w in range(n_win) for p_i in (0, 1)])
        # chunk global id for an edge: need per-call chunk base
        chunks_per_call = np.array([len(s) for s in s_cols_per_call], np.int64)
        call_chunk_base = np.r_[0, np.cumsum(chunks_per_call)][:-1].reshape(n_win, 2)

        e_call_off = call_off[e_win, half]
        e_gpos = e_call_off + e_pos                      # global idx position
        e_chunk = call_chunk_base[e_win, half] + e_pos // P
        e_row = e_pos % P
        e_scol = chunk_scol0[e_chunk] + (e_slot - e_win * WIN) - chunk_col0_flat[e_chunk]

        e_val = (cout[s_local + (0 if phase == 0 else N_U)]
                 * cin[d_local + dst_base]).astype(np.float32)

        for k in range(N_CORES):
            m = e_core == k
            idx_flat = np.zeros(tot_idx, np.int16)
            idx_flat[e_gpos[m]] = s_half_local[m].astype(np.int16)
            sv = np.zeros((P, tot_scols), np.float32)
            sv[e_row[m], e_scol[m]] = e_val[m]
            per_core_idx[k].append(idx_flat)
            per_core_sval[k].append(sv)

    # wrap idx per call into the [16, n/16].T-tiled layout, concat everything
    per_core = []
    for k in range(N_CORES):
        idx_cols = []
        for phase in range(2):
            ph = layout_phases[phase]
            flat = per_core_idx[k][phase]
            off = 0
            for w in range(ph["n_win"]):
                for p_i in (0, 1):
                    nb = ph["windows"][w]["passes"][p_i]["n_blocks"]
                    n = nb * P
                    call = flat[off:off + n]
                    off += n
                    t = call.reshape(n // 16, 16).T      # [16, n/16]
                    idx_cols.append(np.tile(t, (N_CORES, 1)))
        idx_arr = np.concatenate(idx_cols, axis=1)       # [128, tot/16]
        s_arr = np.concatenate(per_core_sval[k], axis=1)  # [128, scols]
        per_core.append({"idx": idx_arr, "s": s_arr, "dsts": per_core_dsts[k]})
    return layout_phases, per_core


# ------------------------------------------------------------------ device code
def _build_nc(sched):
    import concourse.bacc as bacc
    import concourse.bass as bass
    import concourse.mybir as mybir
    from concourse._compat import get_trn_type
    from concourse.library_config import mlp

    nc = bacc.Bacc(get_trn_type() or "TRN2", target_bir_lowering=False, debug=False)
    f32 = mybir.dt.float32
    u_f = nc.dram_tensor("u_f", [N_U, D], f32, kind="ExternalInput")
    v_f = nc.dram_tensor("v_f", [N_V, D], f32, kind="ExternalInput")
    u_w = nc.dram_tensor("u_w", [D, D], f32, kind="ExternalInput")
    v_w = nc.dram_tensor("v_w", [D, D], f32, kind="ExternalInput")

    # totals from schedule
    tot_idx_cols = 0
    tot_scols = 0
    tot_slots = 0
    nblk_max = 0
    for ph in sched:
        for w in ph["windows"]:
            tot_slots += w["n_slots"]
            nb = 0
            for p_i in (0, 1):
                pa = w["passes"][p_i]
                nb += pa["n_blocks"]
                tot_idx_cols += pa["n_blocks"] * P // 16
                tot_scols += sum(c["cols"] for c in pa["chunks"])
            nblk_max = max(nblk_max, nb)

    idx_in = nc.dram_tensor("idx", [P, tot_idx_cols], mybir.dt.int16,
                            kind="ExternalInput")
    s_in = nc.dram_tensor("sval", [P, tot_scols], f32, kind="ExternalInput")
    out = nc.dram_tensor("out", [P, tot_slots], f32, kind="ExternalOutput")

    idx_sb = nc.alloc_sbuf_tensor("idx_sb", [P, tot_idx_cols], mybir.dt.int16)
    m_sb = [nc.alloc_sbuf_tensor(f"m{i}", [P, nblk_max, P], f32) for i in (0, 1)]
    s_sb = [nc.alloc_sbuf_tensor(f"s{i}", [P, 2 * WIN], f32) for i in (0, 1)]
    agg_sb = [nc.alloc_sbuf_tensor(f"agg{i}", [P, WIN], f32) for i in (0, 1)]
    stage_sb = nc.alloc_sbuf_tensor("stage", [P, tot_slots], f32)
    w_sb = [nc.alloc_sbuf_tensor(f"w{i}", [P, D], f32) for i in (0, 1)]

    agg_ps = [nc.alloc_psum_tensor(f"aps{i}", [P, WIN], f32) for i in (0, 1)]
    proj_ps = [nc.alloc_psum_tensor(f"pps{i}", [P, WIN], f32) for i in (0, 1)]

    sem_ld = nc.alloc_semaphore("ld")        # upfront loads
    sem_idx = nc.alloc_semaphore("idxld")    # idx table load
    sem_s = [nc.alloc_semaphore(f"ssem{i}") for i in (0, 1)]
    sem_g = [nc.alloc_semaphore(f"gsem{i}") for i in (0, 1)]
    sem_mm = [nc.alloc_semaphore(f"mmsem{i}") for i in (0, 1)]
    sem_agg = [nc.alloc_semaphore(f"aggsem{i}") for i in (0, 1)]
    sem_proj = [nc.alloc_semaphore(f"projsem{i}") for i in (0, 1)]
    sem_stage = [nc.alloc_semaphore(f"stsem{i}") for i in (0, 1)]

    # flatten windows across phases into one global list
    wlist = []
    icol = 0
    scol = 0
    slot0 = 0
    for phase, ph in enumerate(sched):
        for w in ph["windows"]:
            entry = {"phase": phase, "n_slots": w["n_slots"], "passes": [],
                     "slot0": slot0}
            for p_i in (0, 1):
                pa = w["passes"][p_i]
                nb = pa["n_blocks"]
                entry["passes"].append({
                    "icol": icol, "nb": nb,
                    "chunks": pa["chunks"], "scol": scol,
                })
                icol += nb * P // 16
                scol += sum(c["cols"] for c in pa["chunks"])
            slot0 += w["n_slots"]
            wlist.append(entry)
    NW = len(wlist)

    # counters for sem bookkeeping
    g_cnt = [0, 0]
    s_cnt = [0, 0]
    mm_cnt = [0, 0]
    agg_cnt = [0, 0]
    proj_cnt = [0, 0]
    stage_cnt = [0, 0]

    with nc.Block() as block:
        @block.sync
        def _(sy: bass.BassEngine):
            sy.dma_start(idx_sb[:], idx_in[:]).then_inc(sem_idx, 16)
            sy.dma_start(w_sb[0][:], u_w[:]).then_inc(sem_ld, 16)
            sy.dma_start(w_sb[1][:], v_w[:]).then_inc(sem_ld, 16)
            cnt = [0, 0]
            for wi, went in enumerate(wlist):
                b = wi % 2
                # WAR: S buffer b free after window wi-2's matmuls done
                if wi >= 2:
                    sy.wait_ge(sem_mm[b], cnt[b])
                p0, p1 = went["passes"]
                ncols = (sum(c["cols"] for c in p0["chunks"])
                         + sum(c["cols"] for c in p1["chunks"]))
                sy.dma_start(
                    s_sb[b][:, :ncols], s_in[:, p0["scol"]:p0["scol"] + ncols]
                ).then_inc(sem_s[b], 16)
                cnt[b] = mm_counts[wi]
            # final output
            sy.wait_ge(sem_stage[0], stage_counts[0])
            sy.wait_ge(sem_stage[1], stage_counts[1])
            sy.dma_start(out[:], stage_sb[:]).then_inc(sem_ld, 16)
            sy.wait_ge(sem_ld, 48)

        @block.gpsimd
        def _(gp: bass.BassGpSimd):
            gp.load_library(mlp)
            gp.wait_ge(sem_idx, 16)   # idx loaded
            cnt = [0, 0]
            for wi, went in enumerate(wlist):
                b = wi % 2
                if wi >= 2:
                    gp.wait_ge(sem_mm[b], cnt[b])
                phase = went["phase"]
                if phase == 0:
                    tab_lo, tab_hi = u_f[0:HALF, :], u_f[HALF:N_U, :]
                else:
                    tab_lo, tab_hi = v_f[0:HALF, :], v_f[HALF:N_V, :]
                blk0 = 0
                for p_i, tab in ((0, tab_lo), (1, tab_hi)):
                    pa = went["passes"][p_i]
                    n = pa["nb"] * P
                    if n:
                        gp.dma_gather(
                            m_sb[b][:, blk0:blk0 + pa["nb"], :],
                            tab,
                            idx_sb[:, pa["icol"]:pa["icol"] + n // 16],
                            n, n, D,
                            single_packet=False,
                        ).then_inc(sem_g[b], 16)
                        g_cnt[b] += 16
                    blk0 += pa["nb"]
                cnt[b] = mm_counts[wi]

        @block.tensor
        def _(te):
            g_seen = [0, 0]
            s_seen = [0, 0]
            for wi, went in enumerate(wlist):
                b = wi % 2
                phase = went["phase"]
                # wait gather lo+hi & S stream for this window
                g_seen[b] += 32 if went["passes"][1]["nb"] else 16
                s_seen[b] += 16
                te.wait_ge(sem_g[b], g_seen[b])
                te.wait_ge(sem_s[b], s_seen[b])
                if wi >= 2:
                    te.wait_ge(sem_agg[b], agg_counts_prior[wi])
                ns = went["n_slots"]
                blk0 = 0
                p0scol = went["passes"][0]["scol"]
                last = None
                for p_i in (0, 1):
                    pa = went["passes"][p_i]
                    for ci, ch in enumerate(pa["chunks"]):
                        last = (p_i, ci)
                first = True
                for p_i in (0, 1):
                    pa = went["passes"][p_i]
                    for ci, ch in enumerate(pa["chunks"]):
                        blk = blk0 + ci
                        sc = ch["scol0"] - went["passes"][0]["scol"]
                        mm = te.matmul(
                            out=agg_ps[b][:, ch["col0"]:ch["col0"] + ch["cols"]],
                            lhsT=m_sb[b][:, blk, :],
                            rhs=s_sb[b][:, sc:sc + ch["cols"]],
                            start=first,
                            stop=((p_i, ci) == last),
                        )
                        first = False
                        if (p_i, ci) == last:
                            mm.then_inc(sem_mm[b], 1)
                            mm_cnt[b] += 1
                    blk0 += pa["nb"]
                # projection: wait for vector to copy agg->sbuf
                te.wait_ge(sem_agg[b], agg_counts[wi])
                if wi >= 2:
                    te.wait_ge(sem_stage[b], wi // 2)
                pr = te.matmul(
                    out=proj_ps[b][:, :ns],
                    lhsT=w_sb[phase][:],
                    rhs=agg_sb[b][:, :ns],
                    start=True, stop=True,
                ).then_inc(sem_proj[b], 1)
                proj_cnt[b] += 1

        @block.vector
        def _(ve):
            mm_seen = [0, 0]
            pr_seen = [0, 0]
            for wi, went in enumerate(wlist):
                b = wi % 2
                ns = went["n_slots"]
                mm_seen[b] += 1
                ve.wait_ge(sem_mm[b], mm_seen[b])
                ve.tensor_copy(out=agg_sb[b][:, :ns],
                               in_=agg_ps[b][:, :ns]).then_inc(sem_agg[b], 1)
                agg_cnt[b] += 1
                pr_seen[b] += 1
                ve.wait_ge(sem_proj[b], pr_seen[b])
                ve.tensor_copy(
                    out=stage_sb[:, went["slot0"]:went["slot0"] + ns],
                    in_=proj_ps[b][:, :ns],
                ).then_inc(sem_stage[b], 1)
                stage_cnt[b] += 1

    nc.compile()
    return nc


# pre-computed per-window cumulative targets, filled by kernel() before _build_nc
mm_counts = {}
agg_counts = {}
agg_counts_prior = {}
stage_counts = [0, 0]


def _fill_counts(sched):
    """Cumulative semaphore targets per window (python-side bookkeeping)."""
    wi = 0
    mm_c = [0, 0]
    agg_c = [0, 0]
    stage_c = [0, 0]
    order = []
    for ph in sched:
        for w in ph["windows"]:
            order.append(w)
    for wi, w in enumerate(order):
        b = wi % 2
        mm_c[b] += 1
        mm_counts[wi] = mm_c[b]
        agg_counts_prior[wi] = agg_c[b]  # target before reusing agg bank b
        agg_c[b] += 1
        agg_counts[wi] = agg_c[b]
        stage_c[b] += 1
    stage_counts[0] = stage_c[0]
    stage_counts[1] = stage_c[1]
    return len(order)


# ---------------------------------------------------------------------- kernel
def kernel(u_f, v_f, u_w, v_w, src, dst):
    from concourse.bass_utils import run_bass_kernel_spmd

    src = np.asarray(src)
    dst = np.asarray(dst)
    u_f = np.asarray(u_f, np.float32)
    v_f = np.asarray(v_f, np.float32)

    deg_out = np.bincount(src, minlength=N).astype(np.float32)
    deg_in = np.bincount(dst, minlength=N).astype(np.float32)
    cout = np.maximum(deg_out, 1.0) ** -0.5
    cin = np.maximum(deg_in, 1.0) ** -0.5

    sched, per_core = _build_layout(src, dst, cout, cin)
    _fill_counts(sched)

    nc = _build_nc(sched)
    in_maps = []
    for k in range(N_CORES):
        in_maps.append({
            "u_f": u_f, "v_f": v_f,
            "u_w": np.asarray(u_w, np.float32),
            "v_w": np.asarray(v_w, np.float32),
            "idx": per_core[k]["idx"], "sval": per_core[k]["s"],
        })
    trace = bool(os.environ.get("KERNEL_TRACE"))
    res = run_bass_kernel_spmd(nc, in_maps, core_ids=list(range(N_CORES)),
                               trace=trace)
    if trace:
        print(f"HW exec time: {res.exec_time_ns} ns")
        kernel.last_profile = res.profile_json

    out_full = np.zeros((N, D), np.float32)
    for k in range(N_CORES):
        fm = res.results[k]["out"]            # [128, tot_slots]
        rows = np.ascontiguousarray(fm.T)     # [tot_slots, 128]
        slot0 = 0
        for phase in range(2):
            dsts = per_core[k]["dsts"][phase]
            nslots = len(dsts)
            valid = dsts >= 0
            out_full[dsts[valid]] = rows[slot0:slot0 + nslots][valid]
            slot0 += nslots
    return out_full



# revision 2
# speedup vs baseline: 3.0602x; 3.0602x over previous
"""Bipartite GCN message-passing kernel for 8 Trainium2 NeuronCores.

Math (reference): rst = deg_in^-1/2 * segsum_dst( (node_f @ W_side) * deg_out^-1/2 [src] )
Refactor (projection is linear, graph strictly bipartite):
    rst[d] = ( sum_{e->d} c_e * f_raw[src_e] ) @ W_side(d),
    c_e = deg_out[src]^-1/2 * deg_in[dst]^-1/2  (folded into scatter values on host)

Device pipeline per core (dst slots dealt round-robin by degree rank -> SPMD):
  1. per (window=512 dst slots, table half): dma_gather of bf16 feature rows
     (256B) by src, one gather call per <=32-block segment, round-robin over
     the 4 SWDGE queues so descriptor generation runs on all 4 Q7 core pairs
     concurrently (~4x emission throughput vs single queue).
  2. scatter-matmul: agg_PSUM[128f, 512slot] += M_chunk[128e,128f].T @ S_chunk
     [128e, span] with host-built bf16 S carrying c_e; chunks may straddle
     slots; spans gap-extended so every PSUM column is written.
  3. per-window projection with the side weight (bf16) and fp32 staging out.
Host casts features/weights to bf16, builds idx/S streams, unpermutes output.
"""
import sys
import os

for _p in ("/opt/trn_rl_repo",):
    if _p not in sys.path and os.path.isdir(_p):
        sys.path.insert(0, _p)

import numpy as np

N_U = 50000
N_V = 50000
N = N_U + N_V
D = 128
E = 1600000
N_CORES = 8
HALF = 25000          # int16-safe table window
WIN = 512             # dst slots per PSUM window/bank
P = 128
SEGB = 32             # max gather blocks (128 idx each) per call = 4096 idx
NSLAB = 12            # gather slab ring depth
NQ = 4                # SWDGE queues
SPC = N_U // N_CORES  # 6250 slots per core per phase
NWIN = (SPC + WIN - 1) // WIN   # 13 windows per phase


# ----------------------------------------------------------------- host layout
def _build_layout(src, dst, cout, cin):
    """Canonical (SPMD-identical) schedule + per-core idx/S/output data.

    Returns (sched, per_core).
      sched: {"winpass": [...], "calls": [...], "tot_idx", "tot_scols",
              "smax", "windows": [...]}
      per_core[k]: {"idx": [128, tot_idx//16] i16, "s": [128, tot_scols] f32,
                    "dsts": [phase0 dst map, phase1 dst map]}
    """
    winpass = []           # per (phase, win, pass): dict
    calls = []             # per call: dict
    windows = []           # per global window: {"ns", "slot0", "phase"}
    per_core_idx = [np.zeros(0, np.int16)] * 0

    tot_idx = 0
    tot_scols = 0
    core_idx_parts = [[] for _ in range(N_CORES)]
    core_s_cols = 0
    e_data = []            # per phase: (e_core, e_gpos, e_row, e_scol, e_val, s_half_local)
    per_core_dsts = [[] for _ in range(N_CORES)]

    for phase in range(2):
        if phase == 0:       # dsts are v-nodes, sources u-side
            mask = dst >= N_U
            d_local = dst[mask] - N_U
            s_local = src[mask]
            dst_base = N_U
        else:                # dsts are u-nodes, sources v-side
            mask = dst < N_U
            d_local = dst[mask]
            s_local = src[mask] - N_U
            dst_base = 0
        half = (s_local >= HALF).astype(np.int64)
        s_half_local = (s_local - half * HALF).astype(np.int16)

        a_cnt = np.bincount(d_local[half == 0], minlength=N_U)
        b_cnt = np.bincount(d_local[half == 1], minlength=N_U)

        order = np.lexsort((np.arange(N_U), b_cnt, a_cnt))
        rank = np.empty(N_U, np.int64)
        rank[order] = np.arange(N_U)

        # canonical per-slot degrees = max over cores
        a_mat = np.zeros((N_CORES, SPC), np.int64)
        b_mat = np.zeros((N_CORES, SPC), np.int64)
        dst_mat = np.empty((N_CORES, SPC), np.int64)
        r = np.arange(N_U)
        a_mat[r % N_CORES, r // N_CORES] = a_cnt[order]
        b_mat[r % N_CORES, r // N_CORES] = b_cnt[order]
        dst_mat[r % N_CORES, r // N_CORES] = order + dst_base
        A = a_mat.max(axis=0)
        B = b_mat.max(axis=0)
        for k in range(N_CORES):
            per_core_dsts[k].append(dst_mat[k])

        # canonical chunking / spans / segments per (window, pass)
        pos_base = [np.zeros(SPC, np.int64), np.zeros(SPC, np.int64)]
        wp_meta = [[None, None] for _ in range(NWIN)]
        for w in range(NWIN):
            s0, s1 = w * WIN, min((w + 1) * WIN, SPC)
            nsl = s1 - s0
            if phase == 0 and len(windows) <= w + 0:
                pass
            for p_i, C in enumerate((A, B)):
                Cw = C[s0:s1]
                n = int(Cw.sum())
                assert n > 0
                cum = np.cumsum(Cw)
                pos_base[p_i][s0:s1] = np.r_[0, cum[:-1]]
                nb = (n + P - 1) // P
                # slot (window-local) of each canonical position
                slot_of = np.repeat(np.arange(nsl), Cw)
                chunks = []
                prev_end = -1
                sc = 0
                for kblk in range(nb):
                    lo = kblk * P
                    hi = min((kblk + 1) * P, n) - 1
                    st = min(int(slot_of[lo]), prev_end + 1)
                    en = int(slot_of[hi]) if kblk < nb - 1 else nsl - 1
                    en = max(en, st)
                    chunks.append({"st": st, "en": en, "sc": sc})
                    sc += en - st + 1
                    prev_end = en
                swidth = sc
                segs = []
                for b0 in range(0, nb, SEGB):
                    segs.append((b0, min(SEGB, nb - b0)))
                wp_meta[w][p_i] = {
                    "n": n, "nb": nb, "chunks": chunks, "swidth": swidth,
                    "segs": segs,
                    "idx_off": tot_idx, "scol_off": tot_scols,
                    "phase": phase, "w": w, "p": p_i, "nsl": nsl,
                }
                winpass.append(wp_meta[w][p_i])
                for b0, nbk in segs:
                    calls.append({
                        "phase": phase, "w": w, "p": p_i,
                        "blk0": b0, "nblk": nbk,
                        "icol": (tot_idx + b0 * P) // 16,
                        "n": nbk * P,
                        "wp": len(winpass) - 1,
                    })
                tot_idx += nb * P
                tot_scols += swidth
            windows.append({
                "ns": s1 - s0, "slot0": phase * SPC + s0, "phase": phase,
            })

        # ---- per-core edge placement (vectorized)
        grp = d_local * 2 + half
        sort_i = np.argsort(grp, kind="stable")
        grp_s = grp[sort_i]
        starts = np.r_[0, np.nonzero(np.diff(grp_s))[0] + 1]
        group_id = np.cumsum(np.r_[0, (np.diff(grp_s) != 0).astype(np.int64)])
        within = np.arange(len(grp_s)) - starts[group_id]
        e_rank = np.empty(len(grp), np.int64)
        e_rank[sort_i] = within

        e_rankd = rank[d_local]
        e_core = e_rankd % N_CORES
        e_slot = e_rankd // N_CORES
        e_win = e_slot // WIN
        e_sl_in_win = e_slot - e_win * WIN

        # position within the (win, pass) stream
        pb = np.where(half == 0, pos_base[0][e_slot], pos_base[1][e_slot])
        e_pos = pb + e_rank

        idx_off_map = np.zeros((NWIN, 2), np.int64)
        scol_off_map = np.zeros((NWIN, 2), np.int64)
        for w in range(NWIN):
            for p_i in (0, 1):
                idx_off_map[w, p_i] = wp_meta[w][p_i]["idx_off"]
                scol_off_map[w, p_i] = wp_meta[w][p_i]["scol_off"]

        e_gpos = idx_off_map[e_win, half] + e_pos

        # chunk lookup for scol: chunk = e_pos // P within (win, pass)
        # need chunk span starts: build flat arrays per (win, pass)
        ch_st_flat = {}
        for w in range(NWIN):
            for p_i in (0, 1):
                m = wp_meta[w][p_i]
                ch_st_flat[(w, p_i)] = (
                    np.array([c["st"] for c in m["chunks"]], np.int64),
                    np.array([c["sc"] for c in m["chunks"]], np.int64),
                )
        e_chunk = e_pos // P
        e_scol = np.empty(len(grp), np.int64)
        for w in range(NWIN):
            for p_i in (0, 1):
                m2 = (e_win == w) & (half == p_i)
                if not m2.any():
                    continue
                st_arr, sc_arr = ch_st_flat[(w, p_i)]
                ch = e_chunk[m2]
                e_scol[m2] = (scol_off_map[w, p_i] + sc_arr[ch]
                              + e_sl_in_win[m2] - st_arr[ch])

        e_val = (cout[s_local + (0 if phase == 0 else N_U)]
                 * cin[d_local + dst_base]).astype(np.float32)
        e_data.append((e_core, e_gpos, e_pos % P, e_scol, e_val, s_half_local))

    # ---- build per-core flat arrays
    per_core = []
    for k in range(N_CORES):
        idx_flat = np.zeros(tot_idx, np.int16)
        sval = np.zeros((P, tot_scols), np.float32)
        for (e_core, e_gpos, e_row, e_scol, e_val, shl) in e_data:
            m = e_core == k
            idx_flat[e_gpos[m]] = shl[m]
            sval[e_row[m], e_scol[m]] = e_val[m]
        # wrap idx per call into [16, n/16] tiled x8
        cols = []
        for c in calls:
            wpm = winpass[c["wp"]]
            a = wpm["idx_off"] + c["blk0"] * P
            seg = idx_flat[a:a + c["n"]]
            t = seg.reshape(c["n"] // 16, 16).T
            cols.append(np.tile(t, (N_CORES, 1)))
        idx_arr = np.ascontiguousarray(np.concatenate(cols, axis=1))
        per_core.append({"idx": idx_arr, "s": sval,
                         "dsts": per_core_dsts[k]})

    smax = max(m["swidth"] for m in winpass)
    nbmax = max(min(SEGB, m["nb"]) for m in winpass)
    sched = {"winpass": winpass, "calls": calls, "windows": windows,
             "tot_idx": tot_idx, "tot_scols": tot_scols, "smax": smax,
             "nbmax": nbmax}
    return sched, per_core


# ------------------------------------------------------------------ device code
def _build_nc(sched):
    import concourse.bacc as bacc
    import concourse.bass as bass
    import concourse.mybir as mybir
    from concourse._compat import get_trn_type
    from concourse.library_config import mlp

    nc = bacc.Bacc(get_trn_type() or "TRN2", target_bir_lowering=False,
                   debug=False, num_swdge_queues=NQ)
    f32 = mybir.dt.float32
    bf16 = mybir.dt.bfloat16
    i16 = mybir.dt.int16

    u16 = nc.dram_tensor("u16", [N_U, D], bf16, kind="ExternalInput")
    v16 = nc.dram_tensor("v16", [N_V, D], bf16, kind="ExternalInput")
    uw = nc.dram_tensor("uw", [D, D], bf16, kind="ExternalInput")
    vw = nc.dram_tensor("vw", [D, D], bf16, kind="ExternalInput")

    calls = sched["calls"]
    winpass = sched["winpass"]
    windows = sched["windows"]
    tot_idx = sched["tot_idx"]
    tot_scols = sched["tot_scols"]
    smax = sched["smax"]
    NW = len(windows)
    NC_ = len(calls)
    NWP = len(winpass)

    idx_in = nc.dram_tensor("idx", [P, tot_idx // 16], i16, kind="ExternalInput")
    s_in = nc.dram_tensor("sval", [P, tot_scols], bf16, kind="ExternalInput")
    out = nc.dram_tensor("out", [P, 2 * SPC], f32, kind="ExternalOutput")

    idx_sb = nc.alloc_sbuf_tensor("idx_sb", [P, tot_idx // 16], i16)
    slabs = [nc.alloc_sbuf_tensor(f"m{i}", [P, SEGB, P], bf16)
             for i in range(NSLAB)]
    s_sb = [nc.alloc_sbuf_tensor(f"s{i}", [P, smax], bf16) for i in range(4)]
    agg_sb = [nc.alloc_sbuf_tensor(f"agg{i}", [P, WIN], bf16) for i in (0, 1)]
    stage = [nc.alloc_sbuf_tensor(f"st{i}", [P, WIN], f32) for i in (0, 1)]
    w_sb = [nc.alloc_sbuf_tensor(f"w{i}", [P, D], bf16) for i in (0, 1)]

    agg_ps = [nc.alloc_psum_tensor(f"aps{i}", [P, WIN], f32) for i in (0, 1)]
    proj_ps = [nc.alloc_psum_tensor(f"pps{i}", [P, WIN], f32) for i in (0, 1)]

    sem_idx = nc.alloc_semaphore("idxld")
    sem_ld = nc.alloc_semaphore("wld")
    sem_q = [nc.alloc_semaphore(f"q{i}") for i in range(NQ)]
    sem_s = nc.alloc_semaphore("ssem")
    sem_mm = nc.alloc_semaphore("mmcall")     # +1 per consumed call (tensor)
    sem_mmw = nc.alloc_semaphore("mmwin")     # +1 per window agg done
    sem_agg = nc.alloc_semaphore("aggsem")
    sem_proj = nc.alloc_semaphore("projsem")
    sem_stage = nc.alloc_semaphore("stsem")
    sem_out = nc.alloc_semaphore("outsem")

    # host-side cumulative counts
    # calls per winpass (cumulative), for S-buffer WAR
    calls_cum_wp = []
    cnt = 0
    for i, m in enumerate(winpass):
        cnt += len(m["segs"])
        calls_cum_wp.append(cnt)
    # queue completion targets per call
    q_target = [0] * NC_
    q_cnt = [0] * NQ
    for c_i in range(NC_):
        q = c_i % NQ
        q_cnt[q] += 16
        q_target[c_i] = q_cnt[q]
    # map call -> window index (global), and window -> first/last call
    def win_g(c):
        return c["phase"] * NWIN + c["w"]
    win_first_call = {}
    win_last_call = {}
    for c_i, c in enumerate(calls):
        wg = win_g(c)
        if wg not in win_first_call:
            win_first_call[wg] = c_i
        win_last_call[wg] = c_i

    with nc.Block() as block:
        @block.sync
        def _(sy: bass.BassEngine):
            sy.dma_start(idx_sb[:], idx_in[:]).then_inc(sem_idx, 16)
            sy.dma_start(w_sb[0][:], uw[:]).then_inc(sem_ld, 16)
            sy.dma_start(w_sb[1][:], vw[:]).then_inc(sem_ld, 16)
            for i, m in enumerate(winpass):
                if i >= 4:
                    sy.wait_ge(sem_mm, calls_cum_wp[i - 4])
                sy.dma_start(
                    s_sb[i % 4][:, :m["swidth"]],
                    s_in[:, m["scol_off"]:m["scol_off"] + m["swidth"]],
                ).then_inc(sem_s, 16)
            sy.wait_ge(sem_out, NW * 16)

        @block.gpsimd
        def _(gp: bass.BassGpSimd):
            gp.load_library(mlp)
            gp.wait_ge(sem_idx, 16)
            for c_i, c in enumerate(calls):
                if c_i >= NSLAB:
                    gp.wait_ge(sem_mm, c_i - NSLAB + 1)
                m = winpass[c["wp"]]
                if c["phase"] == 0:
                    tab = u16[0:HALF, :] if c["p"] == 0 else u16[HALF:N_U, :]
                else:
                    tab = v16[0:HALF, :] if c["p"] == 0 else v16[HALF:N_V, :]
                gp.dma_gather(
                    slabs[c_i % NSLAB][:, :c["nblk"], :],
                    tab,
                    idx_sb[:, c["icol"]:c["icol"] + c["n"] // 16],
                    c["n"], c["n"], D,
                    single_packet=False,
                    queue_num=c_i % NQ,
                ).then_inc(sem_q[c_i % NQ], 16)

        @block.tensor
        def _(te):
            te.wait_ge(sem_ld, 32)
            s_seen = {}
            for c_i, c in enumerate(calls):
                m = winpass[c["wp"]]
                wg = win_g(c)
                te.wait_ge(sem_q[c_i % NQ], q_target[c_i])
                if c["wp"] not in s_seen:
                    te.wait_ge(sem_s, 16 * (c["wp"] + 1))
                    s_seen[c["wp"]] = True
                if c_i == win_first_call[wg] and wg >= 2:
                    te.wait_ge(sem_agg, wg - 1)
                b = wg % 2
                first_of_win = c_i == win_first_call[wg] and c["p"] == 0 \
                    and c["blk0"] == 0
                for kb in range(c["nblk"]):
                    blk = c["blk0"] + kb
                    ch = m["chunks"][blk]
                    span = ch["en"] - ch["st"] + 1
                    is_first = first_of_win and kb == 0
                    is_last = (c_i == win_last_call[wg]
                               and kb == c["nblk"] - 1)
                    mm = te.matmul(
                        out=agg_ps[b][:, ch["st"]:ch["en"] + 1],
                        lhsT=slabs[c_i % NSLAB][:, kb, :],
                        rhs=s_sb[c["wp"] % 4][:, ch["sc"]:ch["sc"] + span],
                        start=is_first,
                        stop=is_last,
                    )
                    if is_last:
                        mm.then_inc(sem_mmw, 1)
                te.sem_inc(sem_mm, 1)
                if c_i == win_last_call[wg]:
                    # projection for window wg
                    te.wait_ge(sem_agg, wg + 1)
                    if wg >= 2:
                        te.wait_ge(sem_stage, wg - 1)
                    te.matmul(
                        out=proj_ps[b][:, :windows[wg]["ns"]],
                        lhsT=w_sb[windows[wg]["phase"]][:],
                        rhs=agg_sb[b][:, :windows[wg]["ns"]],
                        start=True, stop=True,
                    ).then_inc(sem_proj, 1)

        @block.vector
        def _(ve):
            for wg in range(NW):
                b = wg % 2
                ns = windows[wg]["ns"]
                ve.wait_ge(sem_mmw, wg + 1)
                ve.tensor_copy(out=agg_sb[b][:, :ns],
                               in_=agg_ps[b][:, :ns]).then_inc(sem_agg, 1)
                ve.wait_ge(sem_proj, wg + 1)
                if wg >= 2:
                    ve.wait_ge(sem_out, 16 * (wg - 1))
                ve.tensor_copy(out=stage[b][:, :ns],
                               in_=proj_ps[b][:, :ns]).then_inc(sem_stage, 1)

        @block.scalar
        def _(sc):
            for wg in range(NW):
                b = wg % 2
                ns = windows[wg]["ns"]
                s0 = windows[wg]["slot0"]
                sc.wait_ge(sem_stage, wg + 1)
                sc.dma_start(out[:, s0:s0 + ns],
                             stage[b][:, :ns]).then_inc(sem_out, 16)

    nc.compile()
    return nc


# ---------------------------------------------------------------------- kernel
def kernel(u_f, v_f, u_w, v_w, src, dst):
    import ml_dtypes
    from concourse.bass_utils import run_bass_kernel_spmd

    src = np.asarray(src)
    dst = np.asarray(dst)
    u_f = np.asarray(u_f, np.float32)
    v_f = np.asarray(v_f, np.float32)

    deg_out = np.bincount(src, minlength=N).astype(np.float32)
    deg_in = np.bincount(dst, minlength=N).astype(np.float32)
    cout = np.maximum(deg_out, 1.0) ** -0.5
    cin = np.maximum(deg_in, 1.0) ** -0.5

    sched, per_core = _build_layout(src, dst, cout, cin)

    nc = _build_nc(sched)
    bf = ml_dtypes.bfloat16
    u16 = u_f.astype(bf)
    v16 = v_f.astype(bf)
    uw16 = np.asarray(u_w, np.float32).astype(bf)
    vw16 = np.asarray(v_w, np.float32).astype(bf)
    in_maps = []
    for k in range(N_CORES):
        in_maps.append({
            "u16": u16, "v16": v16, "uw": uw16, "vw": vw16,
            "idx": per_core[k]["idx"],
            "sval": per_core[k]["s"].astype(bf),
        })
    trace = bool(os.environ.get("KERNEL_TRACE"))
    res = run_bass_kernel_spmd(nc, in_maps, core_ids=list(range(N_CORES)),
                               trace=trace)
    if trace:
        print(f"HW exec time: {res.exec_time_ns} ns")
        kernel.last_profile = res.profile_json

    out_full = np.zeros((N, D), np.float32)
    for k in range(N_CORES):
        fm = res.results[k]["out"]            # [128, 2*SPC] feat-major
        rows = np.ascontiguousarray(fm.T)     # [2*SPC, 128]
        for phase in range(2):
            dsts = per_core[k]["dsts"][phase]
            out_full[dsts] = rows[phase * SPC:(phase + 1) * SPC]
    return out_full


# revision 3
# speedup vs baseline: 3.7454x; 1.2239x over previous
"""Bipartite GCN message-passing kernel for 8 Trainium2 NeuronCores.

Math (reference): rst = deg_in^-1/2 * segsum_dst( (node_f @ W_side) * deg_out^-1/2 [src] )
Refactor (projection is linear, graph strictly bipartite):
    rst[d] = ( sum_{e->d} c_e * f_raw[src_e] ) @ W_side(d),
    c_e = deg_out[src]^-1/2 * deg_in[dst]^-1/2  (folded into scatter values on host)

Device pipeline per core (dst slots dealt round-robin by degree rank -> SPMD):
  1. per (window=512 dst slots, table half): dma_gather of bf16 feature rows
     (256B) by src, one gather call per <=32-block segment, round-robin over
     the 4 SWDGE queues so descriptor generation runs on all 4 Q7 core pairs
     concurrently (~4x emission throughput vs single queue).
  2. scatter-matmul: agg_PSUM[128f, 512slot] += M_chunk[128e,128f].T @ S_chunk
     [128e, span] with host-built bf16 S carrying c_e; chunks may straddle
     slots; spans gap-extended so every PSUM column is written.
  3. per-window projection with the side weight (bf16) and fp32 staging out.
Host casts features/weights to bf16, builds idx/S streams, unpermutes output.
"""
import sys
import os

for _p in ("/opt/trn_rl_repo",):
    if _p not in sys.path and os.path.isdir(_p):
        sys.path.insert(0, _p)

import numpy as np

N_U = 50000
N_V = 50000
N = N_U + N_V
D = 128
E = 1600000
N_CORES = 8
HALF = 25000          # int16-safe table window
WIN = 512             # dst slots per PSUM window/bank
P = 128
SEGB = 16             # max gather blocks (128 idx each) per call = 2048 idx
NSLAB = 12            # gather slab ring depth
NQ = 4                # SWDGE queues
SPC = N_U // N_CORES  # 6250 slots per core per phase
NWIN = (SPC + WIN - 1) // WIN   # 13 windows per phase


# ----------------------------------------------------------------- host layout
def _build_layout(src, dst, cout, cin):
    """Canonical (SPMD-identical) schedule + per-core idx/S/output data.

    Returns (sched, per_core).
      sched: {"winpass": [...], "calls": [...], "tot_idx", "tot_scols",
              "smax", "windows": [...]}
      per_core[k]: {"idx": [128, tot_idx//16] i16, "s": [128, tot_scols] f32,
                    "dsts": [phase0 dst map, phase1 dst map]}
    """
    winpass = []           # per (phase, win, pass): dict
    calls = []             # per call: dict
    windows = []           # per global window: {"ns", "slot0", "phase"}
    per_core_idx = [np.zeros(0, np.int16)] * 0

    tot_idx = 0
    tot_scols = 0
    core_idx_parts = [[] for _ in range(N_CORES)]
    core_s_cols = 0
    e_data = []            # per phase: (e_core, e_gpos, e_row, e_scol, e_val, s_half_local)
    per_core_dsts = [[] for _ in range(N_CORES)]

    for phase in range(2):
        if phase == 0:       # dsts are v-nodes, sources u-side
            mask = dst >= N_U
            d_local = dst[mask] - N_U
            s_local = src[mask]
            dst_base = N_U
        else:                # dsts are u-nodes, sources v-side
            mask = dst < N_U
            d_local = dst[mask]
            s_local = src[mask] - N_U
            dst_base = 0
        half = (s_local >= HALF).astype(np.int64)
        s_half_local = (s_local - half * HALF).astype(np.int16)

        a_cnt = np.bincount(d_local[half == 0], minlength=N_U)
        b_cnt = np.bincount(d_local[half == 1], minlength=N_U)

        order = np.lexsort((np.arange(N_U), b_cnt, a_cnt))
        rank = np.empty(N_U, np.int64)
        rank[order] = np.arange(N_U)

        # canonical per-slot degrees = max over cores
        a_mat = np.zeros((N_CORES, SPC), np.int64)
        b_mat = np.zeros((N_CORES, SPC), np.int64)
        dst_mat = np.empty((N_CORES, SPC), np.int64)
        r = np.arange(N_U)
        a_mat[r % N_CORES, r // N_CORES] = a_cnt[order]
        b_mat[r % N_CORES, r // N_CORES] = b_cnt[order]
        dst_mat[r % N_CORES, r // N_CORES] = order + dst_base
        A = a_mat.max(axis=0)
        B = b_mat.max(axis=0)
        for k in range(N_CORES):
            per_core_dsts[k].append(dst_mat[k])

        # canonical chunking / spans / segments per (window, pass)
        pos_base = [np.zeros(SPC, np.int64), np.zeros(SPC, np.int64)]
        wp_meta = [[None, None] for _ in range(NWIN)]
        for w in range(NWIN):
            s0, s1 = w * WIN, min((w + 1) * WIN, SPC)
            nsl = s1 - s0
            if phase == 0 and len(windows) <= w + 0:
                pass
            for p_i, C in enumerate((A, B)):
                Cw = C[s0:s1]
                n = int(Cw.sum())
                assert n > 0
                cum = np.cumsum(Cw)
                pos_base[p_i][s0:s1] = np.r_[0, cum[:-1]]
                nb = (n + P - 1) // P
                # slot (window-local) of each canonical position
                slot_of = np.repeat(np.arange(nsl), Cw)
                chunks = []
                prev_end = -1
                sc = 0
                for kblk in range(nb):
                    lo = kblk * P
                    hi = min((kblk + 1) * P, n) - 1
                    st = min(int(slot_of[lo]), prev_end + 1)
                    en = int(slot_of[hi]) if kblk < nb - 1 else nsl - 1
                    en = max(en, st)
                    chunks.append({"st": st, "en": en, "sc": sc})
                    sc += en - st + 1
                    prev_end = en
                swidth = sc
                nseg = (nb + SEGB - 1) // SEGB
                base_sz = nb // nseg
                extra = nb - base_sz * nseg
                segs = []
                b0 = 0
                for si in range(nseg):
                    nbk = base_sz + (1 if si < extra else 0)
                    segs.append((b0, nbk))
                    b0 += nbk
                wp_meta[w][p_i] = {
                    "n": n, "nb": nb, "chunks": chunks, "swidth": swidth,
                    "segs": segs,
                    "idx_off": tot_idx, "scol_off": tot_scols,
                    "phase": phase, "w": w, "p": p_i, "nsl": nsl,
                }
                winpass.append(wp_meta[w][p_i])
                for b0, nbk in segs:
                    calls.append({
                        "phase": phase, "w": w, "p": p_i,
                        "blk0": b0, "nblk": nbk,
                        "icol": (tot_idx + b0 * P) // 16,
                        "n": nbk * P,
                        "wp": len(winpass) - 1,
                    })
                tot_idx += nb * P
                tot_scols += swidth
            windows.append({
                "ns": s1 - s0, "slot0": phase * SPC + s0, "phase": phase,
            })

        # ---- per-core edge placement (vectorized)
        grp = d_local * 2 + half
        sort_i = np.argsort(grp, kind="stable")
        grp_s = grp[sort_i]
        starts = np.r_[0, np.nonzero(np.diff(grp_s))[0] + 1]
        group_id = np.cumsum(np.r_[0, (np.diff(grp_s) != 0).astype(np.int64)])
        within = np.arange(len(grp_s)) - starts[group_id]
        e_rank = np.empty(len(grp), np.int64)
        e_rank[sort_i] = within

        e_rankd = rank[d_local]
        e_core = e_rankd % N_CORES
        e_slot = e_rankd // N_CORES
        e_win = e_slot // WIN
        e_sl_in_win = e_slot - e_win * WIN

        # position within the (win, pass) stream
        pb = np.where(half == 0, pos_base[0][e_slot], pos_base[1][e_slot])
        e_pos = pb + e_rank

        idx_off_map = np.zeros((NWIN, 2), np.int64)
        scol_off_map = np.zeros((NWIN, 2), np.int64)
        for w in range(NWIN):
            for p_i in (0, 1):
                idx_off_map[w, p_i] = wp_meta[w][p_i]["idx_off"]
                scol_off_map[w, p_i] = wp_meta[w][p_i]["scol_off"]

        e_gpos = idx_off_map[e_win, half] + e_pos

        # chunk lookup for scol: chunk = e_pos // P within (win, pass)
        # need chunk span starts: build flat arrays per (win, pass)
        ch_st_flat = {}
        for w in range(NWIN):
            for p_i in (0, 1):
                m = wp_meta[w][p_i]
                ch_st_flat[(w, p_i)] = (
                    np.array([c["st"] for c in m["chunks"]], np.int64),
                    np.array([c["sc"] for c in m["chunks"]], np.int64),
                )
        e_chunk = e_pos // P
        e_scol = np.empty(len(grp), np.int64)
        for w in range(NWIN):
            for p_i in (0, 1):
                m2 = (e_win == w) & (half == p_i)
                if not m2.any():
                    continue
                st_arr, sc_arr = ch_st_flat[(w, p_i)]
                ch = e_chunk[m2]
                e_scol[m2] = (scol_off_map[w, p_i] + sc_arr[ch]
                              + e_sl_in_win[m2] - st_arr[ch])

        e_val = (cout[s_local + (0 if phase == 0 else N_U)]
                 * cin[d_local + dst_base]).astype(np.float32)
        e_data.append((e_core, e_gpos, e_pos % P, e_scol, e_val, s_half_local))

    # ---- build per-core flat arrays
    per_core = []
    for k in range(N_CORES):
        idx_flat = np.zeros(tot_idx, np.int16)
        sval = np.zeros((P, tot_scols), np.float32)
        for (e_core, e_gpos, e_row, e_scol, e_val, shl) in e_data:
            m = e_core == k
            idx_flat[e_gpos[m]] = shl[m]
            sval[e_row[m], e_scol[m]] = e_val[m]
        # wrap idx per call into [16, n/16] tiled x8
        cols = []
        for c in calls:
            wpm = winpass[c["wp"]]
            a = wpm["idx_off"] + c["blk0"] * P
            seg = idx_flat[a:a + c["n"]]
            t = seg.reshape(c["n"] // 16, 16).T
            cols.append(np.tile(t, (N_CORES, 1)))
        idx_arr = np.ascontiguousarray(np.concatenate(cols, axis=1))
        per_core.append({"idx": idx_arr, "s": sval,
                         "dsts": per_core_dsts[k]})

    qload = [0] * NQ
    for c in calls:
        q = min(range(NQ), key=lambda i: qload[i])
        qload[q] += c["n"]
        c["q"] = q
    smax = max(m["swidth"] for m in winpass)
    nbmax = max(min(SEGB, m["nb"]) for m in winpass)
    sched = {"winpass": winpass, "calls": calls, "windows": windows,
             "tot_idx": tot_idx, "tot_scols": tot_scols, "smax": smax,
             "nbmax": nbmax}
    return sched, per_core


# ------------------------------------------------------------------ device code
def _build_nc(sched):
    import concourse.bacc as bacc
    import concourse.bass as bass
    import concourse.mybir as mybir
    from concourse._compat import get_trn_type
    from concourse.library_config import mlp

    nc = bacc.Bacc(get_trn_type() or "TRN2", target_bir_lowering=False,
                   debug=False, num_swdge_queues=NQ)
    f32 = mybir.dt.float32
    bf16 = mybir.dt.bfloat16
    i16 = mybir.dt.int16

    u16 = nc.dram_tensor("u16", [N_U, D], bf16, kind="ExternalInput")
    v16 = nc.dram_tensor("v16", [N_V, D], bf16, kind="ExternalInput")
    uw = nc.dram_tensor("uw", [D, D], bf16, kind="ExternalInput")
    vw = nc.dram_tensor("vw", [D, D], bf16, kind="ExternalInput")

    calls = sched["calls"]
    winpass = sched["winpass"]
    windows = sched["windows"]
    tot_idx = sched["tot_idx"]
    tot_scols = sched["tot_scols"]
    smax = sched["smax"]
    NW = len(windows)
    NC_ = len(calls)
    NWP = len(winpass)

    idx_in = nc.dram_tensor("idx", [P, tot_idx // 16], i16, kind="ExternalInput")
    s_in = nc.dram_tensor("sval", [P, tot_scols], bf16, kind="ExternalInput")
    out = nc.dram_tensor("out", [P, 2 * SPC], f32, kind="ExternalOutput")

    idx_sb = nc.alloc_sbuf_tensor("idx_sb", [P, tot_idx // 16], i16)
    slabs = [nc.alloc_sbuf_tensor(f"m{i}", [P, SEGB, P], bf16)
             for i in range(NSLAB)]
    s_sb = [nc.alloc_sbuf_tensor(f"s{i}", [P, smax], bf16) for i in range(4)]
    agg_sb = [nc.alloc_sbuf_tensor(f"agg{i}", [P, WIN], bf16) for i in (0, 1)]
    stage = [nc.alloc_sbuf_tensor(f"st{i}", [P, WIN], f32) for i in (0, 1)]
    w_sb = [nc.alloc_sbuf_tensor(f"w{i}", [P, D], bf16) for i in (0, 1)]

    agg_ps = [nc.alloc_psum_tensor(f"aps{i}", [P, WIN], f32) for i in (0, 1)]
    proj_ps = [nc.alloc_psum_tensor(f"pps{i}", [P, WIN], f32) for i in (0, 1)]

    sem_idx = nc.alloc_semaphore("idxld")
    sem_ld = nc.alloc_semaphore("wld")
    sem_q = [nc.alloc_semaphore(f"q{i}") for i in range(NQ)]
    sem_s = nc.alloc_semaphore("ssem")
    sem_mm = nc.alloc_semaphore("mmcall")     # +1 per consumed call (tensor)
    sem_mmw = nc.alloc_semaphore("mmwin")     # +1 per window agg done
    sem_agg = nc.alloc_semaphore("aggsem")
    sem_proj = nc.alloc_semaphore("projsem")
    sem_stage = nc.alloc_semaphore("stsem")
    sem_out = nc.alloc_semaphore("outsem")

    # host-side cumulative counts
    # calls per winpass (cumulative), for S-buffer WAR
    calls_cum_wp = []
    cnt = 0
    for i, m in enumerate(winpass):
        cnt += len(m["segs"])
        calls_cum_wp.append(cnt)
    # queue completion targets per call
    q_target = [0] * NC_
    q_cnt = [0] * NQ
    for c_i in range(NC_):
        q = calls[c_i]["q"]
        q_cnt[q] += 16
        q_target[c_i] = q_cnt[q]
    # map call -> window index (global), and window -> first/last call
    def win_g(c):
        return c["phase"] * NWIN + c["w"]
    win_first_call = {}
    win_last_call = {}
    for c_i, c in enumerate(calls):
        wg = win_g(c)
        if wg not in win_first_call:
            win_first_call[wg] = c_i
        win_last_call[wg] = c_i

    with nc.Block() as block:
        @block.sync
        def _(sy: bass.BassEngine):
            sy.dma_start(idx_sb[:], idx_in[:]).then_inc(sem_idx, 16)
            sy.dma_start(w_sb[0][:], uw[:]).then_inc(sem_ld, 16)
            sy.dma_start(w_sb[1][:], vw[:]).then_inc(sem_ld, 16)
            for i, m in enumerate(winpass):
                if i >= 4:
                    sy.wait_ge(sem_mm, calls_cum_wp[i - 4])
                sy.dma_start(
                    s_sb[i % 4][:, :m["swidth"]],
                    s_in[:, m["scol_off"]:m["scol_off"] + m["swidth"]],
                ).then_inc(sem_s, 16)
            sy.wait_ge(sem_out, NW * 16)

        @block.gpsimd
        def _(gp: bass.BassGpSimd):
            gp.load_library(mlp)
            gp.wait_ge(sem_idx, 16)
            for c_i, c in enumerate(calls):
                if c_i >= NSLAB:
                    gp.wait_ge(sem_mm, c_i - NSLAB + 1)
                m = winpass[c["wp"]]
                if c["phase"] == 0:
                    tab = u16[0:HALF, :] if c["p"] == 0 else u16[HALF:N_U, :]
                else:
                    tab = v16[0:HALF, :] if c["p"] == 0 else v16[HALF:N_V, :]
                gp.dma_gather(
                    slabs[c_i % NSLAB][:, :c["nblk"], :],
                    tab,
                    idx_sb[:, c["icol"]:c["icol"] + c["n"] // 16],
                    c["n"], c["n"], D,
                    single_packet=False,
                    queue_num=c["q"],
                ).then_inc(sem_q[c["q"]], 16)

        @block.tensor
        def _(te):
            te.wait_ge(sem_ld, 32)
            s_seen = {}
            for c_i, c in enumerate(calls):
                m = winpass[c["wp"]]
                wg = win_g(c)
                te.wait_ge(sem_q[c["q"]], q_target[c_i])
                if c["wp"] not in s_seen:
                    te.wait_ge(sem_s, 16 * (c["wp"] + 1))
                    s_seen[c["wp"]] = True
                if c_i == win_first_call[wg] and wg >= 2:
                    te.wait_ge(sem_agg, wg - 1)
                b = wg % 2
                first_of_win = c_i == win_first_call[wg] and c["p"] == 0 \
                    and c["blk0"] == 0
                for kb in range(c["nblk"]):
                    blk = c["blk0"] + kb
                    ch = m["chunks"][blk]
                    span = ch["en"] - ch["st"] + 1
                    is_first = first_of_win and kb == 0
                    is_last = (c_i == win_last_call[wg]
                               and kb == c["nblk"] - 1)
                    mm = te.matmul(
                        out=agg_ps[b][:, ch["st"]:ch["en"] + 1],
                        lhsT=slabs[c_i % NSLAB][:, kb, :],
                        rhs=s_sb[c["wp"] % 4][:, ch["sc"]:ch["sc"] + span],
                        start=is_first,
                        stop=is_last,
                    )
                    if is_last:
                        mm.then_inc(sem_mmw, 1)
                te.sem_inc(sem_mm, 1)
                if c_i == win_last_call[wg]:
                    # projection for window wg
                    te.wait_ge(sem_agg, wg + 1)
                    if wg >= 2:
                        te.wait_ge(sem_stage, wg - 1)
                    te.matmul(
                        out=proj_ps[b][:, :windows[wg]["ns"]],
                        lhsT=w_sb[windows[wg]["phase"]][:],
                        rhs=agg_sb[b][:, :windows[wg]["ns"]],
                        start=True, stop=True,
                    ).then_inc(sem_proj, 1)

        @block.vector
        def _(ve):
            for wg in range(NW):
                b = wg % 2
                ns = windows[wg]["ns"]
                ve.wait_ge(sem_mmw, wg + 1)
                ve.tensor_copy(out=agg_sb[b][:, :ns],
                               in_=agg_ps[b][:, :ns]).then_inc(sem_agg, 1)
                ve.wait_ge(sem_proj, wg + 1)
                if wg >= 2:
                    ve.wait_ge(sem_out, 16 * (wg - 1))
                ve.tensor_copy(out=stage[b][:, :ns],
                               in_=proj_ps[b][:, :ns]).then_inc(sem_stage, 1)

        @block.scalar
        def _(sc):
            for wg in range(NW):
                b = wg % 2
                ns = windows[wg]["ns"]
                s0 = windows[wg]["slot0"]
                sc.wait_ge(sem_stage, wg + 1)
                sc.dma_start(out[:, s0:s0 + ns],
                             stage[b][:, :ns]).then_inc(sem_out, 16)

    nc.compile()
    return nc


# ---------------------------------------------------------------------- kernel
def kernel(u_f, v_f, u_w, v_w, src, dst):
    import ml_dtypes
    from concourse.bass_utils import run_bass_kernel_spmd

    src = np.asarray(src)
    dst = np.asarray(dst)
    u_f = np.asarray(u_f, np.float32)
    v_f = np.asarray(v_f, np.float32)

    deg_out = np.bincount(src, minlength=N).astype(np.float32)
    deg_in = np.bincount(dst, minlength=N).astype(np.float32)
    cout = np.maximum(deg_out, 1.0) ** -0.5
    cin = np.maximum(deg_in, 1.0) ** -0.5

    sched, per_core = _build_layout(src, dst, cout, cin)

    nc = _build_nc(sched)
    bf = ml_dtypes.bfloat16
    u16 = u_f.astype(bf)
    v16 = v_f.astype(bf)
    uw16 = np.asarray(u_w, np.float32).astype(bf)
    vw16 = np.asarray(v_w, np.float32).astype(bf)
    in_maps = []
    for k in range(N_CORES):
        in_maps.append({
            "u16": u16, "v16": v16, "uw": uw16, "vw": vw16,
            "idx": per_core[k]["idx"],
            "sval": per_core[k]["s"].astype(bf),
        })
    trace = bool(os.environ.get("KERNEL_TRACE"))
    res = run_bass_kernel_spmd(nc, in_maps, core_ids=list(range(N_CORES)),
                               trace=trace)
    if trace:
        print(f"HW exec time: {res.exec_time_ns} ns")
        kernel.last_profile = res.profile_json

    out_full = np.zeros((N, D), np.float32)
    for k in range(N_CORES):
        fm = res.results[k]["out"]            # [128, 2*SPC] feat-major
        rows = np.ascontiguousarray(fm.T)     # [2*SPC, 128]
        for phase in range(2):
            dsts = per_core[k]["dsts"][phase]
            out_full[dsts] = rows[phase * SPC:(phase + 1) * SPC]
    return out_full
